# revision 6
# baseline (speedup 1.0000x reference)
"""Conv2D 3x3 (B=32, C=128, H=W=56 -> OC=256) as a Bass/Tile kernel on 8 NeuronCores.

Strategy: data-parallel over batch (4 images per core), W/b replicated.
The conv is computed as 9 shift-matmuls accumulated in PSUM:
  out[oc, h, w] = sum_{kh,kw} W[oc, :, kh, kw] @ x_pad[:, h+kh, w+kw]
with x zero-padded to 58x58 on the host so every shifted window is a clean
strided view of one SBUF tile. Contraction dim C=128 sits on partitions,
OC=256 is two 128-row output tiles, and the 56x56 output plane is processed
in 7 blocks of 8 rows (N = 8*56 = 448 <= 512, one PSUM bank).

v2 scheduling (vs v1):
- Lead-in DMAs split across BOTH HWDGE rings (qSPDynamicHW via nc.sync and
  qActDynamicHW via nc.scalar) so the first real matmul is gated by one
  x-chunk + one weight tap arriving in parallel (~9.4us) instead of a
  serial chain on one ring (~12.8us).
- HAM warmup right-sized: many small N=128 matmuls that drain just as the
  lead-in data lands, instead of 7 cold N=512 matmuls that kept the PE
  queue busy 4.5us past data-ready.
- Whole-image input DMAs (issue occupancy on a ring is ~650ns regardless
  of size): 28 chunked loads -> 5 loads.
- Output DMAs paired (two 448-col groups -> one 896-col DMA): 58 -> ~32
  issues; final groups split into 224-col quarters across both rings to
  shorten the end-of-kernel DMA drain.

matmul dtype: float16 by default (full PE rate with fast weight load via
FWL; ~3e-4 rel err vs the fp32 reference given this problem's small dynamic
range). Set CONV_MM_DTYPE=f32r/bf16/f32 to switch.
"""

import os

import numpy as np

import concourse.bacc as bacc
import concourse.mybir as mybir
import concourse.tile as tile
from concourse import bass_utils

B, C, H, W_SP = 32, 128, 56, 56
OC, KH, KW = 256, 3, 3
N_CORES = 8
B_PER = B // N_CORES            # 4 images per core
HP, WP = H + 2, W_SP + 2        # zero-padded spatial dims (58x58)
HWP = HP * WP                   # 3364
HWO = H * W_SP                  # 3136
ROWS_PER_TILE = 8               # output rows per matmul tile
N_TILE = ROWS_PER_TILE * W_SP   # 448 (<=512: one PSUM bank)
N_NT = H // ROWS_PER_TILE       # 7
OC_TILES = OC // 128            # 2

_NC_CACHE: dict[str, object] = {}


def _mm_mode() -> str:
    return os.environ.get("CONV_MM_DTYPE", "f16")


def _build_nc(mode: str):
    in_dt = {
        "bf16": mybir.dt.bfloat16,
        "f16": mybir.dt.float16,
        "f32r": mybir.dt.float32r,
        "f32": mybir.dt.float32,
    }[mode]
    n_warm = int(os.environ.get("CONV_WARM", "26"))
    nc = bacc.Bacc(
        "TRN2",
        target_bir_lowering=False,
        debug=False,
        enable_asserts=False,
        num_devices=N_CORES,
    )
    xp = nc.dram_tensor("xp", [B_PER, C, HWP], in_dt, kind="ExternalInput").ap()
    wt = nc.dram_tensor("wt", [C, KH * KW * OC], in_dt, kind="ExternalInput").ap()
    bias = nc.dram_tensor(
        "bias", [128, OC_TILES], mybir.dt.float32, kind="ExternalInput"
    ).ap()
    out = nc.dram_tensor(
        "out", [B_PER, OC, HWO], mybir.dt.float32, kind="ExternalOutput"
    ).ap()

    C0_ROWS = ROWS_PER_TILE + KH - 1   # 10 padded rows (nt=0 of img0)
    C1_ROWS = ROWS_PER_TILE + KH - 1   # 10 padded rows 8..17 (nt=1 of img0)
    REST_R0 = 16                       # img0 rest tile: padded rows 16..57
    REST_ROWS = HP - REST_R0           # 42

    with tile.TileContext(nc) as tc:
        with (
            tc.tile_pool(name="xin", bufs=1) as xpool0,
            tc.tile_pool(name="ximg", bufs=3) as xipool,
            tc.tile_pool(name="wpool", bufs=1) as wpool,
            tc.tile_pool(name="bpool", bufs=1) as bpool,
            tc.tile_pool(name="opool", bufs=3) as opool,
            tc.tile_pool(name="osmall", bufs=4) as ospool,
            tc.tile_pool(name="psum", bufs=4, space="PSUM") as pspool,
        ):
            # --- lead-in DMAs, both rings in parallel ---------------------
            # sync ring: x rows 0..9 of img0, then rows 16..57 of img0.
            # scalar ring: weight taps (finest first), x rows 8..17, imgs 1-2.
            # gpsimd (SWDGE): bias.
            wsb = wpool.tile([C, KH * KW, OC], in_dt, tag="wsb")
            wtv = wt.rearrange("c (k m) -> c k m", m=OC)
            xv0 = xp[0].rearrange("c (h w) -> c h w", w=WP)

            xc0 = xpool0.tile([C, C0_ROWS, WP], in_dt, tag="xc0")
            nc.sync.dma_start(xc0[:], xv0[:, :C0_ROWS, :])
            nc.scalar.dma_start(wsb[:, 0, :], wtv[:, 0, :])
            xc1 = xpool0.tile([C, C1_ROWS, WP], in_dt, tag="xc1")
            nc.scalar.dma_start(
                xc1[:], xv0[:, ROWS_PER_TILE : ROWS_PER_TILE + C1_ROWS, :]
            )
            xr0 = xpool0.tile([C, REST_ROWS, WP], in_dt, tag="xr0")
            nc.sync.dma_start(xr0[:], xv0[:, REST_R0:, :])
            nc.scalar.dma_start(wsb[:, 1:KW, :], wtv[:, 1:KW, :])
            for kg in range(1, KH):
                nc.scalar.dma_start(
                    wsb[:, kg * KW : (kg + 1) * KW, :],
                    wtv[:, kg * KW : (kg + 1) * KW, :],
                )
            bsb = bpool.tile([128, OC_TILES], mybir.dt.float32, tag="bsb")
            nc.gpsimd.dma_start(bsb[:], bias[:])

            ximg = [None] * B_PER
            for img in range(1, min(3, B_PER)):
                ximg[img] = xipool.tile(
                    [C, HP, WP], in_dt, tag="xi", name=f"xi{img}"
                )
                nc.scalar.dma_start(
                    ximg[img][:], xp[img].rearrange("c (h w) -> c h w", w=WP)
                )

            # --- HAM warm-up ---------------------------------------------
            # The PE clock-gate needs ~3.4us of sustained matmul activity to
            # lift to 2.4 GHz. Issue many small cold matmuls that drain just
            # as the lead-in DMAs complete, so the real stream starts warm
            # and is never queued behind a long warmup matmul.
            if n_warm > 0:
                wu = wpool.tile([C, 128], in_dt, tag="wu")
                nc.gpsimd.memset(wu[:], 0.0)
                psw = pspool.tile([128, N_TILE], mybir.dt.float32, tag="ps")
                for i in range(n_warm):
                    nc.tensor.matmul(
                        psw[:, :128],
                        wu[:],
                        wu[:],
                        start=(i == 0),
                        stop=(i == n_warm - 1),
                    )

            # --- main loop -----------------------------------------------
            def rhs_view(img, nt, sr, kh, nr):
                r = nt * ROWS_PER_TILE + sr + kh
                if img == 0:
                    if nt == 0:
                        return xc0[:, r : r + nr, :]
                    if nt == 1:
                        return xc1[:, r - ROWS_PER_TILE : r - ROWS_PER_TILE + nr, :]
                    return xr0[:, r - REST_R0 : r - REST_R0 + nr, :]
                return ximg[img][:, r : r + nr, :]

            # paired output tiles: ot[oc_t] holds two consecutive nt groups
            for img in range(B_PER):
                if img == 1 and B_PER > 3:
                    ximg[3] = xipool.tile(
                        [C, HP, WP], in_dt, tag="xi", name="xi3"
                    )
                    nc.scalar.dma_start(
                        ximg[3][:], xp[3].rearrange("c (h w) -> c h w", w=WP)
                    )
                last_img = img == B_PER - 1
                ot_pair = [None, None]
                for nt in range(N_NT):
                    # tail shaping: quarters for the very last group, singles
                    # for the two groups before it so the final transfers
                    # retire quickly after the last matmul
                    quarter = last_img and nt == N_NT - 1
                    single = (not quarter) and (
                        nt == N_NT - 1 or (last_img and nt >= N_NT - 3)
                    )
                    for oc_t in range(OC_TILES):
                        subs = [(0, 4), (4, 4)] if quarter else [(0, ROWS_PER_TILE)]
                        for sr, nr in subs:
                            n_free = nr * W_SP
                            ps = pspool.tile(
                                [128, N_TILE], mybir.dt.float32, tag="ps"
                            )
                            for ki in range(KH * KW):
                                kh, kw = divmod(ki, KW)
                                nc.tensor.matmul(
                                    ps[:, :n_free],
                                    wsb[:, ki, oc_t * 128 : (oc_t + 1) * 128],
                                    rhs_view(img, nt, sr, kh, nr)[
                                        :, :, kw : kw + W_SP
                                    ],
                                    start=(ki == 0),
                                    stop=(ki == KH * KW - 1),
                                )
                            ocs = slice(oc_t * 128, (oc_t + 1) * 128)
                            if quarter:
                                # 224-col quarters, DMA'd immediately on
                                # alternating rings to shorten the drain
                                ot = ospool.tile(
                                    [128, N_TILE // 2], mybir.dt.float32, tag="os"
                                )
                                nc.scalar.activation(
                                    ot[:, :n_free],
                                    ps[:, :n_free],
                                    mybir.ActivationFunctionType.Identity,
                                    bias=bsb[:, oc_t : oc_t + 1],
                                )
                                col0 = nt * N_TILE + sr * W_SP
                                eng = nc.sync if oc_t == 0 else nc.scalar
                                eng.dma_start(
                                    out[img, ocs, col0 : col0 + n_free],
                                    ot[:, :n_free],
                                )
                            elif single:
                                ot = ospool.tile(
                                    [128, N_TILE], mybir.dt.float32, tag="os1"
                                )
                                nc.scalar.activation(
                                    ot[:],
                                    ps[:],
                                    mybir.ActivationFunctionType.Identity,
                                    bias=bsb[:, oc_t : oc_t + 1],
                                )
                                col0 = nt * N_TILE
                                eng = nc.sync if oc_t == 0 else nc.scalar
                                eng.dma_start(
                                    out[img, ocs, col0 : col0 + N_TILE], ot[:]
                                )
                            else:
                                lo = nt % 2 == 0
                                if lo:
                                    ot_pair[oc_t] = opool.tile(
                                        [128, 2 * N_TILE],
                                        mybir.dt.float32,
                                        tag="ot",
                                        name=f"ot{oc_t}",
                                    )
                                ot = ot_pair[oc_t]
                                half = 0 if lo else N_TILE
                                nc.scalar.activation(
                                    ot[:, half : half + N_TILE],
                                    ps[:],
                                    mybir.ActivationFunctionType.Identity,
                                    bias=bsb[:, oc_t : oc_t + 1],
                                )
                                if not lo:
                                    col0 = (nt - 1) * N_TILE
                                    nc.sync.dma_start(
                                        out[img, ocs, col0 : col0 + 2 * N_TILE],
                                        ot[:],
                                    )
    nc.compile()
    return nc


def _get_nc(mode: str):
    nc = _NC_CACHE.get(mode)
    if nc is None:
        nc = _build_nc(mode)
        _NC_CACHE[mode] = nc
    return nc


def kernel(x: np.ndarray, W: np.ndarray, b: np.ndarray) -> np.ndarray:
    mode = _mm_mode()
    x = np.asarray(x, dtype=np.float32)
    W = np.asarray(W, dtype=np.float32)
    b = np.asarray(b, dtype=np.float32)

    if mode == "bf16":
        import ml_dtypes

        in_np_dt = ml_dtypes.bfloat16
    elif mode == "f16":
        in_np_dt = np.float16
    else:
        in_np_dt = np.float32

    # Host-side layout prep: zero-pad x spatially, put the conv taps of W
    # into [tap, C, OC] (lhsT layout), stripe bias to [128, OC_TILES].
    xp = np.zeros((B, C, HP, WP), dtype=in_np_dt)
    xp[:, :, 1:-1, 1:-1] = x
    xp = xp.reshape(N_CORES, B_PER, C, HWP)
    # wt[c, k*OC + oc] = W[oc, c*9 + k]  (lhsT tap blocks, contiguous per c)
    wt = np.ascontiguousarray(
        W.reshape(OC, C, KH * KW).transpose(1, 2, 0).reshape(C, KH * KW * OC)
    ).astype(in_np_dt)
    bias = np.ascontiguousarray(b.reshape(OC_TILES, 128).T).astype(np.float32)

    nc = _get_nc(mode)
    in_maps = [
        {"xp": np.ascontiguousarray(xp[i]), "wt": wt, "bias": bias}
        for i in range(N_CORES)
    ]
    trace = os.environ.get("CONV_TRACE", "") not in ("", "0")
    try:
        res = bass_utils.run_bass_kernel_spmd(
            nc,
            in_maps,
            core_ids=list(range(N_CORES)),
            trace=trace,
        )
    except Exception:
        # transient device wedges (NRT_EXEC_UNIT_UNRECOVERABLE) have been
        # observed once; a fresh dispatch usually recovers
        import time

        time.sleep(2.0)
        res = bass_utils.run_bass_kernel_spmd(
            nc,
            in_maps,
            core_ids=list(range(N_CORES)),
            trace=trace,
        )
    kernel._last_results = res  # for test harness introspection
    out = np.stack([res.results[i]["out"] for i in range(N_CORES)])
    return out.reshape(B, OC, H, W_SP)


# revision 11
# speedup vs baseline: 1.0266x; 1.0266x over previous
"""Conv2D 3x3 (B=32, C=128, H=W=56 -> OC=256) as a Bass/Tile kernel on 8 NeuronCores.

Strategy: data-parallel over batch (4 images per core), W/b replicated.
The conv is computed as 9 shift-matmuls accumulated in PSUM:
  out[oc, h, w] = sum_{kh,kw} W[oc, :, kh, kw] @ x_pad[:, h+kh, w+kw]
with x zero-padded to 58x58 on the host so every shifted window is a clean
strided view of one SBUF tile. Contraction dim C=128 sits on partitions,
OC=256 is two 128-row output tiles, and the 56x56 output plane is processed
in 7 blocks of 8 rows (N = 8*56 = 448 <= 512, one PSUM bank).

v2 scheduling (vs v1):
- Lead-in DMAs split across BOTH HWDGE rings (qSPDynamicHW via nc.sync and
  qActDynamicHW via nc.scalar) so the first real matmul is gated by one
  x-chunk + one weight tap arriving in parallel (~9.4us) instead of a
  serial chain on one ring (~12.8us).
- HAM warmup right-sized: many small N=128 matmuls that drain just as the
  lead-in data lands, instead of 7 cold N=512 matmuls that kept the PE
  queue busy 4.5us past data-ready.
- Whole-image input DMAs (issue occupancy on a ring is ~650ns regardless
  of size): 28 chunked loads -> 5 loads.
- Output DMAs paired (two 448-col groups -> one 896-col DMA): 58 -> ~32
  issues; final groups split into 224-col quarters across both rings to
  shorten the end-of-kernel DMA drain.

matmul dtype: float16 by default (full PE rate with fast weight load via
FWL; ~3e-4 rel err vs the fp32 reference given this problem's small dynamic
range). Set CONV_MM_DTYPE=f32r/bf16/f32 to switch.
"""

import os

import numpy as np

import concourse.bacc as bacc
import concourse.mybir as mybir
import concourse.tile as tile
from concourse import bass_utils

B, C, H, W_SP = 32, 128, 56, 56
OC, KH, KW = 256, 3, 3
N_CORES = 8
B_PER = B // N_CORES            # 4 images per core
HP, WP = H + 2, W_SP + 2        # zero-padded spatial dims (58x58)
HWP = HP * WP                   # 3364
HWO = H * W_SP                  # 3136
ROWS_PER_TILE = 8               # output rows per matmul tile
N_TILE = ROWS_PER_TILE * W_SP   # 448 (<=512: one PSUM bank)
N_NT = H // ROWS_PER_TILE       # 7
OC_TILES = OC // 128            # 2

_NC_CACHE: dict[str, object] = {}


def _mm_mode() -> str:
    return os.environ.get("CONV_MM_DTYPE", "f16")


def _build_nc(mode: str):
    in_dt = {
        "bf16": mybir.dt.bfloat16,
        "f16": mybir.dt.float16,
        "f32r": mybir.dt.float32r,
        "f32": mybir.dt.float32,
    }[mode]
    n_warm = int(os.environ.get("CONV_WARM", "38"))
    nc = bacc.Bacc(
        "TRN2",
        target_bir_lowering=False,
        debug=False,
        enable_asserts=False,
        num_devices=N_CORES,
    )
    xp = nc.dram_tensor("xp", [B_PER, C, HWP], in_dt, kind="ExternalInput").ap()
    wt = nc.dram_tensor("wt", [C, KH * KW * OC], in_dt, kind="ExternalInput").ap()
    bias = nc.dram_tensor(
        "bias", [128, OC_TILES], mybir.dt.float32, kind="ExternalInput"
    ).ap()
    out = nc.dram_tensor(
        "out", [B_PER, OC, HWO], mybir.dt.float32, kind="ExternalOutput"
    ).ap()

    C0_ROWS = ROWS_PER_TILE + KH - 1   # 10 padded rows (nt=0 of img0)
    C1_ROWS = ROWS_PER_TILE + KH - 1   # 10 padded rows 8..17 (nt=1 of img0)
    REST_R0 = 16                       # img0 rest tile: padded rows 16..57
    REST_ROWS = HP - REST_R0           # 42

    with tile.TileContext(nc) as tc:
        with (
            tc.tile_pool(name="xin", bufs=1) as xpool0,
            tc.tile_pool(name="ximg", bufs=3) as xipool,
            tc.tile_pool(name="wpool", bufs=1) as wpool,
            tc.tile_pool(name="bpool", bufs=1) as bpool,
            tc.tile_pool(name="opool", bufs=3) as opool,
            tc.tile_pool(name="osmall", bufs=4) as ospool,
            tc.tile_pool(name="psum", bufs=4, space="PSUM") as pspool,
        ):
            # --- HAM warm-up (memset first so it isn't queued behind the
            # SWDGE bias DMA on the gpsimd queue) --------------------------
            wu = wpool.tile([C, 128], in_dt, tag="wu")
            if n_warm > 0:
                nc.gpsimd.memset(wu[:], 0.0)

            # --- lead-in DMAs, both rings in parallel ---------------------
            # scalar(ACT) ring leads with the two tensors gating matmul k=0
            # (xc0 + tap0); sync ring carries the taps gating k=1..8; bulk
            # (img0 rest, imgs 1-3) follows on both. gpsimd (SWDGE): bias.
            wsb = wpool.tile([C, KH * KW, OC], in_dt, tag="wsb")
            wtv = wt.rearrange("c (k m) -> c k m", m=OC)
            xv0 = xp[0].rearrange("c (h w) -> c h w", w=WP)

            xc0 = xpool0.tile([C, C0_ROWS, WP], in_dt, tag="xc0")
            nc.scalar.dma_start(xc0[:], xv0[:, :C0_ROWS, :])
            nc.sync.dma_start(wsb[:, 1:KW, :], wtv[:, 1:KW, :])
            nc.scalar.dma_start(wsb[:, 0, :], wtv[:, 0, :])
            for kg in range(1, KH):
                nc.sync.dma_start(
                    wsb[:, kg * KW : (kg + 1) * KW, :],
                    wtv[:, kg * KW : (kg + 1) * KW, :],
                )
            xc1 = xpool0.tile([C, C1_ROWS, WP], in_dt, tag="xc1")
            nc.scalar.dma_start(
                xc1[:], xv0[:, ROWS_PER_TILE : ROWS_PER_TILE + C1_ROWS, :]
            )
            xr0 = xpool0.tile([C, REST_ROWS, WP], in_dt, tag="xr0")
            nc.sync.dma_start(xr0[:], xv0[:, REST_R0:, :])
            bsb = bpool.tile([128, OC_TILES], mybir.dt.float32, tag="bsb")
            nc.gpsimd.dma_start(bsb[:], bias[:])

            ximg = [None] * B_PER
            for img in range(1, min(3, B_PER)):
                ximg[img] = xipool.tile(
                    [C, HP, WP], in_dt, tag="xi", name=f"xi{img}"
                )
                nc.scalar.dma_start(
                    ximg[img][:], xp[img].rearrange("c (h w) -> c h w", w=WP)
                )

            # The PE clock-gate needs ~3.4us of sustained matmul activity to
            # lift to 2.4 GHz. Issue many small cold matmuls that drain just
            # as the lead-in DMAs complete, so the real stream starts warm
            # and is never queued behind a long warmup matmul.
            if n_warm > 0:
                psw = pspool.tile([128, N_TILE], mybir.dt.float32, tag="ps")
                for i in range(n_warm):
                    nc.tensor.matmul(
                        psw[:, :128],
                        wu[:],
                        wu[:],
                        start=(i == 0),
                        stop=(i == n_warm - 1),
                    )

            # --- main loop -----------------------------------------------
            def rhs_view(img, nt, sr, kh, nr):
                r = nt * ROWS_PER_TILE + sr + kh
                if img == 0:
                    if nt == 0:
                        return xc0[:, r : r + nr, :]
                    if nt == 1:
                        return xc1[:, r - ROWS_PER_TILE : r - ROWS_PER_TILE + nr, :]
                    return xr0[:, r - REST_R0 : r - REST_R0 + nr, :]
                return ximg[img][:, r : r + nr, :]

            # paired output tiles: ot[oc_t] holds two consecutive nt groups
            for img in range(B_PER):
                if img == 1 and B_PER > 3:
                    ximg[3] = xipool.tile(
                        [C, HP, WP], in_dt, tag="xi", name="xi3"
                    )
                    nc.scalar.dma_start(
                        ximg[3][:], xp[3].rearrange("c (h w) -> c h w", w=WP)
                    )
                last_img = img == B_PER - 1
                ot_pair = [None, None]
                for nt in range(N_NT):
                    # tail shaping: quarters for the very last group, singles
                    # for the two groups before it so the final transfers
                    # retire quickly after the last matmul
                    quarter = last_img and nt == N_NT - 1
                    single = (not quarter) and (
                        nt == N_NT - 1 or (last_img and nt >= N_NT - 3)
                    )
                    for oc_t in range(OC_TILES):
                        subs = [(0, 4), (4, 4)] if quarter else [(0, ROWS_PER_TILE)]
                        for sr, nr in subs:
                            n_free = nr * W_SP
                            ps = pspool.tile(
                                [128, N_TILE], mybir.dt.float32, tag="ps"
                            )
                            for ki in range(KH * KW):
                                kh, kw = divmod(ki, KW)
                                nc.tensor.matmul(
                                    ps[:, :n_free],
                                    wsb[:, ki, oc_t * 128 : (oc_t + 1) * 128],
                                    rhs_view(img, nt, sr, kh, nr)[
                                        :, :, kw : kw + W_SP
                                    ],
                                    start=(ki == 0),
                                    stop=(ki == KH * KW - 1),
                                )
                            ocs = slice(oc_t * 128, (oc_t + 1) * 128)
                            if quarter:
                                # 224-col quarters, DMA'd immediately on
                                # alternating rings to shorten the drain
                                ot = ospool.tile(
                                    [128, N_TILE // 2], mybir.dt.float32, tag="os"
                                )
                                nc.vector.tensor_scalar_add(
                                    ot[:, :n_free],
                                    ps[:, :n_free],
                                    bsb[:, oc_t : oc_t + 1],
                                )
                                col0 = nt * N_TILE + sr * W_SP
                                eng = nc.sync if oc_t == 0 else nc.scalar
                                eng.dma_start(
                                    out[img, ocs, col0 : col0 + n_free],
                                    ot[:, :n_free],
                                )
                            elif single:
                                ot = ospool.tile(
                                    [128, N_TILE], mybir.dt.float32, tag="os1"
                                )
                                nc.vector.tensor_scalar_add(
                                    ot[:], ps[:], bsb[:, oc_t : oc_t + 1]
                                )
                                col0 = nt * N_TILE
                                eng = nc.sync if oc_t == 0 else nc.scalar
                                eng.dma_start(
                                    out[img, ocs, col0 : col0 + N_TILE], ot[:]
                                )
                            else:
                                lo = nt % 2 == 0
                                if lo:
                                    ot_pair[oc_t] = opool.tile(
                                        [128, 2 * N_TILE],
                                        mybir.dt.float32,
                                        tag="ot",
                                        name=f"ot{oc_t}",
                                    )
                                ot = ot_pair[oc_t]
                                half = 0 if lo else N_TILE
                                nc.vector.tensor_scalar_add(
                                    ot[:, half : half + N_TILE],
                                    ps[:],
                                    bsb[:, oc_t : oc_t + 1],
                                )
                                if not lo:
                                    col0 = (nt - 1) * N_TILE
                                    nc.sync.dma_start(
                                        out[img, ocs, col0 : col0 + 2 * N_TILE],
                                        ot[:],
                                    )
    nc.compile()
    return nc


def _get_nc(mode: str):
    nc = _NC_CACHE.get(mode)
    if nc is None:
        nc = _build_nc(mode)
        _NC_CACHE[mode] = nc
    return nc


def kernel(x: np.ndarray, W: np.ndarray, b: np.ndarray) -> np.ndarray:
    mode = _mm_mode()
    x = np.asarray(x, dtype=np.float32)
    W = np.asarray(W, dtype=np.float32)
    b = np.asarray(b, dtype=np.float32)

    if mode == "bf16":
        import ml_dtypes

        in_np_dt = ml_dtypes.bfloat16
    elif mode == "f16":
        in_np_dt = np.float16
    else:
        in_np_dt = np.float32

    # Host-side layout prep: zero-pad x spatially, put the conv taps of W
    # into [tap, C, OC] (lhsT layout), stripe bias to [128, OC_TILES].
    xp = np.zeros((B, C, HP, WP), dtype=in_np_dt)
    xp[:, :, 1:-1, 1:-1] = x
    xp = xp.reshape(N_CORES, B_PER, C, HWP)
    # wt[c, k*OC + oc] = W[oc, c*9 + k]  (lhsT tap blocks, contiguous per c)
    wt = np.ascontiguousarray(
        W.reshape(OC, C, KH * KW).transpose(1, 2, 0).reshape(C, KH * KW * OC)
    ).astype(in_np_dt)
    bias = np.ascontiguousarray(b.reshape(OC_TILES, 128).T).astype(np.float32)

    nc = _get_nc(mode)
    in_maps = [
        {"xp": np.ascontiguousarray(xp[i]), "wt": wt, "bias": bias}
        for i in range(N_CORES)
    ]
    trace = os.environ.get("CONV_TRACE", "") not in ("", "0")
    try:
        res = bass_utils.run_bass_kernel_spmd(
            nc,
            in_maps,
            core_ids=list(range(N_CORES)),
            trace=trace,
        )
    except Exception:
        # transient device wedges (NRT_EXEC_UNIT_UNRECOVERABLE) have been
        # observed once; a fresh dispatch usually recovers
        import time

        time.sleep(2.0)
        res = bass_utils.run_bass_kernel_spmd(
            nc,
            in_maps,
            core_ids=list(range(N_CORES)),
            trace=trace,
        )
    kernel._last_results = res  # for test harness introspection
    out = np.stack([res.results[i]["out"] for i in range(N_CORES)])
    return out.reshape(B, OC, H, W_SP)
